# revision 1
# baseline (speedup 1.0000x reference)
"""Causal self-attention (B=4, T=2048, C=1024, H=16, D=64) on 8 Trainium2
NeuronCores.

Sharding: core c = (batch b = c//2, head-group g = c%2 of 8 heads).
Each core computes q/k/v projections for its 8 heads, causal flash-style
attention in S^T = [tk, tq] layout (softmax denominators via a ones-row
appended to V; exp on ScalarE; causal mask via GpSimd affine_select
triangular zeroing of P after the exp; lower-left tile skipping), then a
partial o_proj. Host sums the two head-group partials per batch.

Engine split: TensorE matmuls; ScalarE exp; DVE psum evacuation +
reciprocal + normalize mul; GpSimd mask-zeroing, 1/denominator partition
broadcast, and the first x-chunk DMA queue (parallel with the weight DMAs
on the sync queue so the first matmul group starts ~3us in).

Precision: fp16 end-to-end on TensorE (fp16 weights get a separate
pipelined Ldweights, unlike self-loading fp32r which serializes a ~107ns
weight load into every matmul), fp32 PSUM accumulation, fp16 y partials
summed in fp32 on host (validated ~7e-4 rel err vs fp32 reference).
"""

from contextlib import ExitStack

import numpy as np

import concourse.tile as tile
from concourse import bacc, mybir
from concourse.bass_utils import run_bass_kernel_spmd

F32 = mybir.dt.float32
FP16 = mybir.dt.float16
EXP = mybir.ActivationFunctionType.Exp

B, T, C, NHEAD, D = 4, 2048, 1024, 16, 64
H = 8                      # heads per core
HD = H * D                 # 512
NT = T // 128              # 16 tk tiles
NJ = T // 512              # 4 tq chunks
NC = C // 128              # 8 contraction chunks
NM = HD // 128             # 4 qT/kT partition tiles
NYN = C // 512             # 2 o_proj N chunks
DJ = 4                     # tk tiles per tq chunk


def build_nc(loop_k=0, stages="ABC"):
    nc = bacc.Bacc("TRN2", target_bir_lowering=False, debug=False,
                   enable_asserts=False, num_devices=8)

    xT = nc.dram_tensor("xT", [C, T], FP16, kind="ExternalInput").ap()
    wqT = nc.dram_tensor("wqT", [C, HD], FP16, kind="ExternalInput").ap()
    wkT = nc.dram_tensor("wkT", [C, HD], FP16, kind="ExternalInput").ap()
    wvT = nc.dram_tensor("wvT", [C, HD], FP16, kind="ExternalInput").ap()
    woT = nc.dram_tensor("woT", [HD, C], FP16, kind="ExternalInput").ap()
    y = nc.dram_tensor("y", [T, C], FP16, kind="ExternalOutput").ap()

    with tile.TileContext(nc) as tc:
        with ExitStack() as ctx:
            if loop_k:
                ctx.enter_context(tc.For_i(0, loop_k, 1))
            _body(tc, xT, wqT, wkT, wvT, woT, y, stages)
    nc.compile()
    return nc


def _body(tc, xT, wqT, wkT, wvT, woT, y, stages="ABC"):
    nc = tc.nc
    use_B = "B" in stages
    use_C = "C" in stages
    with ExitStack() as ctx:
        ctx.enter_context(nc.allow_low_precision(reason="fp32r/fp16 pipeline"))
        pers = ctx.enter_context(tc.tile_pool(name="pers", bufs=1))
        qT = [pers.tile([128, T], FP16, tag=f"qT{m}", name=f"qT{m}")
              for m in range(NM)]
        kT = [pers.tile([128, T], FP16, tag=f"kT{m}", name=f"kT{m}")
              for m in range(NM)]
        Vs = [pers.tile([128, H * 65], FP16, tag=f"Vs{t}", name=f"Vs{t}")
              for t in range(NT)]
        OTP = [pers.tile([128, T], FP16, tag=f"OTP{p}", name=f"OTP{p}")
               for p in range(H // 2)]
        wpool = ctx.enter_context(tc.tile_pool(name="wpool", bufs=1))
        xpool = ctx.enter_context(tc.tile_pool(name="xpool", bufs=16))
        wopool = ctx.enter_context(tc.tile_pool(name="wopool", bufs=1))
        ppool = ctx.enter_context(tc.tile_pool(name="ppool", bufs=12))
        rpool = ctx.enter_context(tc.tile_pool(name="rpool", bufs=4))
        psS2 = ctx.enter_context(tc.tile_pool(name="psS2", bufs=2,
                                              space="PSUM"))
        psA = ctx.enter_context(tc.tile_pool(name="psA", bufs=2, space="PSUM"))
        psO = ctx.enter_context(tc.tile_pool(name="psO", bufs=2, space="PSUM"))
        ypool = ctx.enter_context(tc.tile_pool(name="ypool", bufs=4))

        wq = [wpool.tile([128, HD], FP16, tag=f"wq{k}", name=f"wq{k}")
              for k in range(NC)]
        wk = [wpool.tile([128, HD], FP16, tag=f"wk{k}", name=f"wk{k}")
              for k in range(NC)]
        wv = [wpool.tile([128, HD], FP16, tag=f"wv{k}", name=f"wv{k}")
              for k in range(NC)]
        # interleave wq with the first x chunk so the first matmul group can
        # start as soon as wq[k] + xt0[k] land (weights-then-x would stall PE
        # ~25us at kernel start)
        xt0 = []
        for k in range(NC):
            nc.sync.dma_start(wq[k][:], wqT[128 * k:128 * k + 128, :])
            t_ = xpool.tile([128, 512], FP16, tag="xt", name="xt")
            nc.gpsimd.dma_start(t_[:], xT[128 * k:128 * k + 128, 0:512])
            xt0.append(t_)
        for k in range(NC):
            nc.sync.dma_start(wk[k][:], wkT[128 * k:128 * k + 128, :])
        for k in range(NC):
            nc.sync.dma_start(wv[k][:], wvT[128 * k:128 * k + 128, :])
        WoPh = [wopool.tile([128, C], FP16, tag=f"Wo{p}", name=f"Wo{p}")
                for p in range(H // 2)]
        for p in range(H // 2):
            nc.sync.dma_start(WoPh[p][:], woT[128 * p:128 * p + 128, :])

        def emit_normalize(O_ps, h, j):
            # 1/denominator, broadcast across the 64 feature partitions via
            # the (otherwise idle) GpSimd engine, then scale on DVE
            rt = rpool.tile([1, 512], FP16, tag="r", name="rt")
            nc.vector.reciprocal(rt[0:1, :], O_ps[64:65, :])
            Rs = rpool.tile([64, 512], FP16, tag="Rs", name="Rs")
            nc.gpsimd.partition_broadcast(Rs[:], rt[0:1, :])
            cols = slice(512 * j, 512 * j + 512)
            if h % 2 == 0:
                nc.vector.tensor_mul(OTP[h // 2][0:64, cols],
                                     O_ps[0:64, :], Rs[:])
            else:
                ot_tmp = rpool.tile([64, 512], FP16, tag="ot", name="ot_tmp")
                nc.vector.tensor_mul(ot_tmp[:], O_ps[0:64, :], Rs[:])
                nc.sync.dma_start(OTP[h // 2][64:128, cols], ot_tmp[:])

        # ---- stage A emitters: per chunk j, 12 matmul groups ----
        xts = {}

        xts[0] = xt0

        def a_group(j, grp):
            if j not in xts:
                tsl = slice(512 * j, 512 * j + 512)
                xt = []
                for k in range(NC):
                    t_ = xpool.tile([128, 512], FP16, tag="xt", name="xt")
                    # sync queue: the Pool queue would head-of-line block
                    # these behind affine_selects/broadcasts waiting on exps
                    nc.sync.dma_start(t_[:], xT[128 * k:128 * k + 128, tsl])
                    xt.append(t_)
                xts[j] = xt
            xt = xts[j]
            tsl = slice(512 * j, 512 * j + 512)
            if grp < 8:
                dst, w, m = ((qT, wq, grp) if grp < 4 else (kT, wk, grp - 4))
                ps = psA.tile([128, 512], F32, tag="A", name="psA")
                for k in range(NC):
                    nc.tensor.matmul(ps[:], w[k][:, 128 * m:128 * m + 128],
                                     xt[k][:], start=(k == 0),
                                     stop=(k == NC - 1))
                nc.vector.tensor_copy(dst[m][:, tsl], ps[:])
            else:
                tt = grp - 8
                t_idx = 4 * j + tt
                ps = psA.tile([128, 512], F32, tag="A", name="psV")
                for k in range(NC):
                    nc.tensor.matmul(ps[:], xt[k][:, 128 * tt:128 * tt + 128],
                                     wv[k][:], start=(k == 0),
                                     stop=(k == NC - 1))
                dst_ap = Vs[t_idx][:].rearrange("p (h e) -> p h e", e=65)
                nc.vector.tensor_copy(
                    dst_ap[:, :, 0:64],
                    ps[:].rearrange("p (h e) -> p h e", e=64))
                nc.vector.memset(dst_ap[:, :, 64:65], 1.0)

        c_count = [0]

        def c_tile(m, n):
            y_ps = psA.tile([128, 512], F32, tag="A", name="yps")
            for p in range(H // 2):
                nc.tensor.matmul(
                    y_ps[:], OTP[p][:, 128 * m:128 * m + 128],
                    WoPh[p][:, 512 * n:512 * n + 512],
                    start=(p == 0), stop=(p == H // 2 - 1))
            ysb = ypool.tile([128, 512], FP16, tag="y", name="ysb")
            c_count[0] += 1
            nc.vector.tensor_copy(ysb[:], y_ps[:])
            nc.sync.dma_start(
                y[128 * m:128 * m + 128, 512 * n:512 * n + 512], ysb[:])

        a_todo = {j: list(range(12)) for j in range(NJ)}
        c_todo = []
        c_wait = {j: [(m, n) for m in range(4 * j, 4 * j + 4)
                      for n in range(NYN)] for j in range(NJ)}
        norm_done = {j: 0 for j in range(NJ)}

        def emit_filler(j_next, k):
            # prefer A-groups for the next chunk, then ready C-tiles
            while k > 0:
                if a_todo.get(j_next):
                    a_group(j_next, a_todo[j_next].pop(0))
                elif c_todo:
                    c_tile(*c_todo.pop(0))
                else:
                    return
                k -= 1

        for g in range(12):
            a_group(0, a_todo[0].pop(0))
        pending = []

        def pop_normalize():
            if pending:
                O_ps, h, j = pending.pop(0)
                emit_normalize(O_ps, h, j)
                norm_done[j] += 1
                if use_C and norm_done[j] == H:
                    c_todo.extend(c_wait.pop(j))

        for j in (range(NJ) if use_B else []):
            for hp in range(H // 2):
                h2 = (2 * hp, 2 * hp + 1)
                kTh, qTh = kT[hp], qT[hp]
                i_max = DJ * j + DJ - 1
                Ps = []
                for i in range(i_max + 1):
                    mloc = i - DJ * j
                    off = 128 * mloc if mloc > 0 else 0
                    w = 512 - off
                    S2 = psS2.tile([128, 1024], F32, tag="S2", name="S2")
                    diag = mloc >= 0
                    for idx in range(2):
                        pb = 64 * idx
                        nc.tensor.matmul(
                            S2[:, 512 * idx + off:512 * idx + off + w],
                            kTh[pb:pb + 64, 128 * i:128 * i + 128],
                            qTh[pb:pb + 64, 512 * j + off:512 * j + off + w],
                            start=True, stop=True)
                    P2 = ppool.tile([128, 1024], FP16, tag="P", name="P")
                    nc.scalar.activation(P2[:, 0:1024 - off],
                                         S2[:, off:1024], EXP, scale=0.125)
                    if diag:
                        # causal mask: zero P2 where tk > tq in the diagonal
                        # 128-block of both heads (cols 0:128 and 512:640),
                        # on the otherwise-idle GpSimd engine
                        P2r = P2[:].rearrange("p (b c) -> p b c",
                                              b=2)[:, :, 0:128]
                        nc.gpsimd.affine_select(
                            out=P2r, in_=P2r,
                            compare_op=mybir.AluOpType.is_ge,
                            fill=0.0, base=0,
                            pattern=[[0, 2], [1, 128]],
                            channel_multiplier=-1)
                    Ps.append((P2, off, w))
                    if i % 2 == 1:
                        emit_filler(j + 1, 1)
                emit_filler(j + 1, 2)
                # last head-pair of the last chunk: do the odd head (whose
                # OTP write needs a SBUF->SBUF DMA) first, so that DMA
                # latency overlaps the even head's work instead of gating
                # the final o_proj tiles
                last = (j == NJ - 1 and hp == H // 2 - 1)
                for idx in ((1, 0) if last else (0, 1)):
                    pop_normalize()
                    O_ps = psO.tile([65, 512], F32, tag="O", name="Ops")
                    h = h2[idx]
                    for i, (P2, off, w) in enumerate(Ps):
                        rhs = (P2[:, 0:w] if idx == 0
                               else P2[:, 512:512 + w])
                        nc.tensor.matmul(
                            O_ps[:, off:off + w],
                            Vs[i][:, 65 * h:65 * h + 65], rhs,
                            start=(i == 0), stop=(i == i_max))
                    pending.append((O_ps, h, j))
        while pending:
            pop_normalize()
        if use_C:
            while c_todo or any(c_wait.values()):
                if not c_todo:
                    for j in sorted(list(c_wait)):
                        c_todo.extend(c_wait.pop(j))
                c_tile(*c_todo.pop(0))
        else:
            dummy = ypool.tile([128, 512], FP16, tag="y", name="ysb")
            nc.vector.memset(dummy[:], 0.0)
            nc.sync.dma_start(y[0:128, 0:512], dummy[:])


_NC_CACHE = {}


def _get_nc(loop_k=0, stages="ABC"):
    key = (loop_k, stages)
    if key not in _NC_CACHE:
        _NC_CACHE[key] = build_nc(loop_k, stages)
    return _NC_CACHE[key]


def make_in_maps(x, Wq, Wk, Wv, Wo):
    x = np.asarray(x, dtype=np.float32)
    Wq = np.asarray(Wq, dtype=np.float32)
    Wk = np.asarray(Wk, dtype=np.float32)
    Wv = np.asarray(Wv, dtype=np.float32)
    Wo = np.asarray(Wo, dtype=np.float32)
    xTs = [np.ascontiguousarray(x[b].T).astype(np.float16) for b in range(B)]
    in_maps = []
    for c in range(8):
        b, g = c // 2, c % 2
        sl = slice(HD * g, HD * g + HD)
        in_maps.append({
            "xT": xTs[b],
            "wqT": np.ascontiguousarray(Wq[sl, :].T).astype(np.float16),
            "wkT": np.ascontiguousarray(Wk[sl, :].T).astype(np.float16),
            "wvT": np.ascontiguousarray(Wv[sl, :].T).astype(np.float16),
            "woT": np.ascontiguousarray(Wo[:, sl].T).astype(np.float16),
        })
    return in_maps


def kernel(x, Wq, Wk, Wv, Wo):
    nc = _get_nc()
    in_maps = make_in_maps(x, Wq, Wk, Wv, Wo)
    res = run_bass_kernel_spmd(nc, in_maps, core_ids=list(range(8)))
    out = np.empty((B, T, C), dtype=np.float32)
    for b in range(B):
        out[b] = (res.results[2 * b]["y"].astype(np.float32)
                  + res.results[2 * b + 1]["y"].astype(np.float32))
    return out



# revision 2
# speedup vs baseline: 1.4540x; 1.4540x over previous
"""Causal self-attention (B=4, T=2048, C=1024, H=16, D=64) on 8 Trainium2
NeuronCores.

Sharding: core c = (batch b = c//2, head-group g = c%2 of 8 heads).
Each core computes q/k/v projections for its 8 heads, causal flash-style
attention in S^T = [tk, tq] layout (softmax denominators via a ones-row
appended to V; exp on ScalarE; causal mask via GpSimd affine_select
triangular zeroing of P after the exp; lower-left tile skipping), then a
partial o_proj. Host sums the two head-group partials per batch.

DMA design: all DRAM tensors are pre-laid-out on the host to match the
SBUF tile geometry ([128 partitions, wide free dim], 8KB+ contiguous
per-partition runs), so each logical transfer is ONE big DMA near peak
bandwidth instead of 8-32 descriptor-dominated 128KB DMAs (measured
~4.8us fixed+completion cost per small DMA on HW, which dominated the
baseline at ~540us/iter of DMA overhead).

Engine split: TensorE matmuls; ScalarE exp; DVE psum evacuation +
reciprocal + normalize mul; GpSimd mask-zeroing, 1/denominator partition
broadcast, and the x-chunk DMA queue.

Precision: fp16 end-to-end on TensorE, fp32 PSUM accumulation, fp16 y
partials summed in fp32 on host (validated ~7e-4 rel err vs fp32
reference).
"""

from contextlib import ExitStack

import numpy as np

import concourse.tile as tile
from concourse import bacc, mybir
from concourse.bass_utils import run_bass_kernel_spmd

F32 = mybir.dt.float32
FP16 = mybir.dt.float16
EXP = mybir.ActivationFunctionType.Exp

B, T, C, NHEAD, D = 4, 2048, 1024, 16, 64
H = 8                      # heads per core
HD = H * D                 # 512
NT = T // 128              # 16 tk tiles
NJ = T // 512              # 4 tq chunks
NC = C // 128              # 8 contraction chunks
NM = HD // 128             # 4 qT/kT partition tiles
NYN = C // 512             # 2 o_proj N chunks
DJ = 4                     # tk tiles per tq chunk


def build_nc(loop_k=0, stages="ABC"):
    nc = bacc.Bacc("TRN2", target_bir_lowering=False, debug=False,
                   enable_asserts=False, num_devices=8)

    # host-relaid tensors: [128, k*512] with contiguous per-partition runs
    xW = nc.dram_tensor("xW", [128, NJ * 4096], FP16, kind="ExternalInput").ap()
    wqW = nc.dram_tensor("wqW", [128, 4096], FP16, kind="ExternalInput").ap()
    wkW = nc.dram_tensor("wkW", [128, 4096], FP16, kind="ExternalInput").ap()
    wvW = nc.dram_tensor("wvW", [128, 4096], FP16, kind="ExternalInput").ap()
    woW = nc.dram_tensor("woW", [128, 4096], FP16, kind="ExternalInput").ap()
    yW = nc.dram_tensor("yW", [128, 16384], FP16, kind="ExternalOutput").ap()

    with tile.TileContext(nc) as tc:
        with ExitStack() as ctx:
            if loop_k:
                ctx.enter_context(tc.For_i(0, loop_k, 1))
            _body(tc, xW, wqW, wkW, wvW, woW, yW, stages)
    nc.compile()
    return nc


def _body(tc, xW, wqW, wkW, wvW, woW, yW, stages="ABC"):
    nc = tc.nc
    use_B = "B" in stages
    use_C = "C" in stages
    with ExitStack() as ctx:
        ctx.enter_context(nc.allow_low_precision(reason="fp32r/fp16 pipeline"))
        pers = ctx.enter_context(tc.tile_pool(name="pers", bufs=1))
        qT = [pers.tile([128, T], FP16, tag=f"qT{m}", name=f"qT{m}")
              for m in range(NM)]
        kT = [pers.tile([128, T], FP16, tag=f"kT{m}", name=f"kT{m}")
              for m in range(NM)]
        Vs = [pers.tile([128, H * 65], FP16, tag=f"Vs{t}", name=f"Vs{t}")
              for t in range(NT)]
        # all 4 head-pairs' O^T partials in one allocation so the odd-head
        # partition-shift DMA can batch 4 heads per chunk into one transfer
        OTP = pers.tile([128, (H // 2) * T], FP16, tag="OTP", name="OTP")
        wpool = ctx.enter_context(tc.tile_pool(name="wpool", bufs=1))
        xpool = ctx.enter_context(tc.tile_pool(name="xpool", bufs=2))
        ppool = ctx.enter_context(tc.tile_pool(name="ppool", bufs=12))
        rpool = ctx.enter_context(tc.tile_pool(name="rpool", bufs=4))
        otpool = ctx.enter_context(tc.tile_pool(name="otpool", bufs=2))
        psS2 = ctx.enter_context(tc.tile_pool(name="psS2", bufs=2,
                                              space="PSUM"))
        psA = ctx.enter_context(tc.tile_pool(name="psA", bufs=2, space="PSUM"))
        psO = ctx.enter_context(tc.tile_pool(name="psO", bufs=2, space="PSUM"))
        ypool = ctx.enter_context(tc.tile_pool(name="ypool", bufs=2))

        # one big DMA per weight tensor (128 descriptors x 8KB, near-peak)
        wq = wpool.tile([128, 4096], FP16, tag="wq", name="wq")
        nc.sync.dma_start(wq[:], wqW)
        xts = {}
        xts[0] = xpool.tile([128, 4096], FP16, tag="xt", name="xt0")
        nc.sync.dma_start(xts[0][:], xW[:, 0:4096])
        wk = wpool.tile([128, 4096], FP16, tag="wk", name="wk")
        nc.sync.dma_start(wk[:], wkW)
        wv = wpool.tile([128, 4096], FP16, tag="wv", name="wv")
        nc.sync.dma_start(wv[:], wvW)
        Wo = wpool.tile([128, 4096], FP16, tag="Wo", name="Wo")
        nc.sync.dma_start(Wo[:], woW)

        def emit_normalize(O_ps, h, j):
            # 1/denominator, broadcast across the 64 feature partitions via
            # the (otherwise idle) GpSimd engine, then scale on DVE
            hp = h // 2
            rt = rpool.tile([1, 512], FP16, tag="r", name="rt")
            nc.vector.reciprocal(rt[0:1, :], O_ps[64:65, :])
            Rs = rpool.tile([64, 512], FP16, tag="Rs", name="Rs")
            nc.gpsimd.partition_broadcast(Rs[:], rt[0:1, :])
            cols = slice(T * hp + 512 * j, T * hp + 512 * j + 512)
            if h % 2 == 0:
                nc.vector.tensor_mul(OTP[0:64, cols], O_ps[0:64, :], Rs[:])
            else:
                nc.vector.tensor_mul(ot_big[j % 2][:, 512 * hp:512 * hp + 512],
                                     O_ps[0:64, :], Rs[:])

        # per-chunk batch of the 4 odd heads' normalized output, DMA'd into
        # OTP partitions 64:128 in one SBUF->SBUF transfer per chunk
        ot_big = [otpool.tile([64, 2048], FP16, tag="otb", name=f"otb{i}")
                  for i in range(2)]
        odd_done = {j: 0 for j in range(NJ)}

        def finish_odd(j):
            # scalar queue: keeps the sync queue free for the next
            # iteration's weight/x prefetch (FIFO per queue)
            dst = OTP[64:128, :].rearrange("p (hp t) -> p hp t", hp=H // 2)
            nc.scalar.dma_start(
                dst[:, :, 512 * j:512 * j + 512],
                ot_big[j % 2][:].rearrange("p (hp c) -> p hp c", hp=H // 2))

        # ---- stage A emitters: per chunk j, 12 matmul groups ----
        def a_group(j, grp):
            if j not in xts:
                xt_ = xpool.tile([128, 4096], FP16, tag="xt", name=f"xt{j}")
                nc.sync.dma_start(xt_[:], xW[:, 4096 * j:4096 * j + 4096])
                xts[j] = xt_
            xt = xts[j]
            tsl = slice(512 * j, 512 * j + 512)
            if grp < 8:
                dst, w, m = ((qT, wq, grp) if grp < 4 else (kT, wk, grp - 4))
                ps = psA.tile([128, 512], F32, tag="A", name="psA")
                for k in range(NC):
                    nc.tensor.matmul(
                        ps[:], w[:, 512 * k + 128 * m:512 * k + 128 * m + 128],
                        xt[:, 512 * k:512 * k + 512], start=(k == 0),
                        stop=(k == NC - 1))
                nc.vector.tensor_copy(dst[m][:, tsl], ps[:])
            else:
                tt = grp - 8
                t_idx = 4 * j + tt
                ps = psA.tile([128, 512], F32, tag="A", name="psV")
                for k in range(NC):
                    nc.tensor.matmul(
                        ps[:], xt[:, 512 * k + 128 * tt:512 * k + 128 * tt + 128],
                        wv[:, 512 * k:512 * k + 512], start=(k == 0),
                        stop=(k == NC - 1))
                dst_ap = Vs[t_idx][:].rearrange("p (h e) -> p h e", e=65)
                nc.vector.tensor_copy(
                    dst_ap[:, :, 0:64],
                    ps[:].rearrange("p (h e) -> p h e", e=64))
                nc.vector.memset(dst_ap[:, :, 64:65], 1.0)

        y_done = {}

        def c_tile(m, n):
            j = m // 4
            y_ps = psA.tile([128, 512], F32, tag="A", name="yps")
            for p in range(H // 2):
                nc.tensor.matmul(
                    y_ps[:], OTP[:, T * p + 128 * m:T * p + 128 * m + 128],
                    Wo[:, 1024 * p + 512 * n:1024 * p + 512 * n + 512],
                    start=(p == 0), stop=(p == H // 2 - 1))
            if j not in y_done:
                y_done[j] = [ypool.tile([128, 4096], FP16, tag="y",
                                        name=f"ysb{j}"), 0]
            ysb, _ = y_done[j]
            mo = m - 4 * j
            nc.vector.tensor_copy(
                ysb[:, 1024 * mo + 512 * n:1024 * mo + 512 * n + 512], y_ps[:])
            y_done[j][1] += 1
            if y_done[j][1] == 8:
                nc.scalar.dma_start(yW[:, 4096 * j:4096 * j + 4096], ysb[:])

        a_todo = {j: list(range(12)) for j in range(NJ)}
        c_todo = []
        c_wait = {j: [(m, n) for m in range(4 * j, 4 * j + 4)
                      for n in range(NYN)] for j in range(NJ)}
        norm_done = {j: 0 for j in range(NJ)}

        def emit_filler(j_next, k):
            # prefer A-groups for the next chunk, then ready C-tiles
            while k > 0:
                if a_todo.get(j_next):
                    a_group(j_next, a_todo[j_next].pop(0))
                elif c_todo:
                    c_tile(*c_todo.pop(0))
                else:
                    return
                k -= 1

        for g in range(12):
            a_group(0, a_todo[0].pop(0))
        pending = []

        def pop_normalize():
            if pending:
                O_ps, h, j = pending.pop(0)
                emit_normalize(O_ps, h, j)
                norm_done[j] += 1
                if h % 2 == 1:
                    odd_done[j] += 1
                    if odd_done[j] == H // 2:
                        finish_odd(j)
                if use_C and norm_done[j] == H:
                    c_todo.extend(c_wait.pop(j))

        for j in (range(NJ) if use_B else []):
            for hp in range(H // 2):
                h2 = (2 * hp, 2 * hp + 1)
                kTh, qTh = kT[hp], qT[hp]
                i_max = DJ * j + DJ - 1
                Ps = []
                for i in range(i_max + 1):
                    mloc = i - DJ * j
                    off = 128 * mloc if mloc > 0 else 0
                    w = 512 - off
                    S2 = psS2.tile([128, 1024], F32, tag="S2", name="S2")
                    diag = mloc >= 0
                    for idx in range(2):
                        pb = 64 * idx
                        nc.tensor.matmul(
                            S2[:, 512 * idx + off:512 * idx + off + w],
                            kTh[pb:pb + 64, 128 * i:128 * i + 128],
                            qTh[pb:pb + 64, 512 * j + off:512 * j + off + w],
                            start=True, stop=True)
                    P2 = ppool.tile([128, 1024], FP16, tag="P", name="P")
                    nc.scalar.activation(P2[:, 0:1024 - off],
                                         S2[:, off:1024], EXP, scale=0.125)
                    if diag:
                        # causal mask: zero P2 where tk > tq in the diagonal
                        # 128-block of both heads (cols 0:128 and 512:640),
                        # on the otherwise-idle GpSimd engine
                        P2r = P2[:].rearrange("p (b c) -> p b c",
                                              b=2)[:, :, 0:128]
                        nc.gpsimd.affine_select(
                            out=P2r, in_=P2r,
                            compare_op=mybir.AluOpType.is_ge,
                            fill=0.0, base=0,
                            pattern=[[0, 2], [1, 128]],
                            channel_multiplier=-1)
                    Ps.append((P2, off, w))
                    if i % 2 == 1:
                        emit_filler(j + 1, 1)
                emit_filler(j + 1, 2)
                # last head-pair of the last chunk: do the odd head (whose
                # OTP write needs a SBUF->SBUF DMA) first, so that DMA
                # latency overlaps the even head's work instead of gating
                # the final o_proj tiles
                last = (j == NJ - 1 and hp == H // 2 - 1)
                for idx in ((1, 0) if last else (0, 1)):
                    pop_normalize()
                    O_ps = psO.tile([65, 512], F32, tag="O", name="Ops")
                    h = h2[idx]
                    for i, (P2, off, w) in enumerate(Ps):
                        rhs = (P2[:, 0:w] if idx == 0
                               else P2[:, 512:512 + w])
                        nc.tensor.matmul(
                            O_ps[:, off:off + w],
                            Vs[i][:, 65 * h:65 * h + 65], rhs,
                            start=(i == 0), stop=(i == i_max))
                    pending.append((O_ps, h, j))
        while pending:
            pop_normalize()
        if use_C:
            while c_todo or any(c_wait.values()):
                if not c_todo:
                    for j in sorted(list(c_wait)):
                        c_todo.extend(c_wait.pop(j))
                c_tile(*c_todo.pop(0))
        else:
            dummy = ypool.tile([128, 4096], FP16, tag="y", name="ydum")
            nc.vector.memset(dummy[:, 0:512], 0.0)
            nc.sync.dma_start(yW[:, 0:512], dummy[:, 0:512])


_NC_CACHE = {}


def _get_nc(loop_k=0, stages="ABC"):
    key = (loop_k, stages)
    if key not in _NC_CACHE:
        _NC_CACHE[key] = build_nc(loop_k, stages)
    return _NC_CACHE[key]


def make_in_maps(x, Wq, Wk, Wv, Wo):
    x = np.asarray(x, dtype=np.float32)
    Wq = np.asarray(Wq, dtype=np.float32)
    Wk = np.asarray(Wk, dtype=np.float32)
    Wv = np.asarray(Wv, dtype=np.float32)
    Wo = np.asarray(Wo, dtype=np.float32)

    def relay_w(Wslice):
        # [512, 1024] -> wT [1024, 512] -> [8(k), 128(p), 512] -> [p, k*512]
        wT = Wslice.T.reshape(8, 128, 512).transpose(1, 0, 2).reshape(128, 4096)
        return np.ascontiguousarray(wT).astype(np.float16)

    def relay_wo(Wslice):
        # Wo[:, sl].T = [512, 1024] -> [4(g), 128(p), 1024] -> [p, g*1024]
        wT = Wslice.T.reshape(4, 128, 1024).transpose(1, 0, 2).reshape(128, 4096)
        return np.ascontiguousarray(wT).astype(np.float16)

    xWs = []
    for b in range(B):
        # xT [1024(c), 2048(t)] -> [8(k),128(p),4(j),512(tc)] -> [p, j, k, tc]
        xT = x[b].T.reshape(8, 128, 4, 512).transpose(1, 2, 0, 3)
        xWs.append(np.ascontiguousarray(xT.reshape(128, 16384))
                   .astype(np.float16))

    in_maps = []
    for c in range(8):
        b, g = c // 2, c % 2
        sl = slice(HD * g, HD * g + HD)
        in_maps.append({
            "xW": xWs[b],
            "wqW": relay_w(Wq[sl, :]),
            "wkW": relay_w(Wk[sl, :]),
            "wvW": relay_w(Wv[sl, :]),
            "woW": relay_wo(Wo[:, sl]),
        })
    return in_maps


def kernel(x, Wq, Wk, Wv, Wo):
    nc = _get_nc()
    in_maps = make_in_maps(x, Wq, Wk, Wv, Wo)
    res = run_bass_kernel_spmd(nc, in_maps, core_ids=list(range(8)))
    out = np.empty((B, T, C), dtype=np.float32)
    for b in range(B):
        # yW [128, 16*1024] -> y [2048, 1024]
        ys = []
        for cid in (2 * b, 2 * b + 1):
            yw = res.results[cid]["yW"].astype(np.float32)
            ys.append(yw.reshape(128, 16, 1024).transpose(1, 0, 2)
                      .reshape(T, C))
        out[b] = ys[0] + ys[1]
    return out


# revision 3
# speedup vs baseline: 1.5229x; 1.0474x over previous
"""Causal self-attention (B=4, T=2048, C=1024, H=16, D=64) on 8 Trainium2
NeuronCores.

Sharding: core c = (batch b = c//2, head-group g = c%2 of 8 heads).
Each core computes q/k/v projections for its 8 heads, causal flash-style
attention in S^T = [tk, tq] layout (softmax denominators via a ones-row
appended to V; exp on ScalarE; causal mask via GpSimd affine_select
triangular zeroing of P after the exp; lower-left tile skipping), then a
partial o_proj. Host sums the two head-group partials per batch.

DMA design: all DRAM tensors are pre-laid-out on the host to match the
SBUF tile geometry ([128 partitions, wide free dim], 8KB+ contiguous
per-partition runs), so each logical transfer is ONE big DMA near peak
bandwidth instead of 8-32 descriptor-dominated 128KB DMAs (measured
~4.8us fixed+completion cost per small DMA on HW, which dominated the
baseline at ~540us/iter of DMA overhead).

Engine split: TensorE matmuls; ScalarE exp; DVE psum evacuation +
reciprocal + normalize mul; GpSimd mask-zeroing, 1/denominator partition
broadcast, and the x-chunk DMA queue.

Precision: fp16 end-to-end on TensorE, fp32 PSUM accumulation, fp16 y
partials summed in fp32 on host (validated ~7e-4 rel err vs fp32
reference).
"""

from contextlib import ExitStack

import numpy as np

import concourse.tile as tile
from concourse import bacc, mybir
from concourse.bass_utils import run_bass_kernel_spmd

F32 = mybir.dt.float32
FP16 = mybir.dt.float16
EXP = mybir.ActivationFunctionType.Exp

B, T, C, NHEAD, D = 4, 2048, 1024, 16, 64
H = 8                      # heads per core
HD = H * D                 # 512
NT = T // 128              # 16 tk tiles
NJ = T // 512              # 4 tq chunks
NC = C // 128              # 8 contraction chunks
NM = HD // 128             # 4 qT/kT partition tiles
NYN = C // 512             # 2 o_proj N chunks
DJ = 4                     # tk tiles per tq chunk


def build_nc(loop_k=0, stages="ABC"):
    nc = bacc.Bacc("TRN2", target_bir_lowering=False, debug=False,
                   enable_asserts=False, num_devices=8)

    # host-relaid tensors: [128, k*512] with contiguous per-partition runs
    xW = nc.dram_tensor("xW", [128, NJ * 4096], FP16, kind="ExternalInput").ap()
    wqW = nc.dram_tensor("wqW", [128, 4096], FP16, kind="ExternalInput").ap()
    wkW = nc.dram_tensor("wkW", [128, 4096], FP16, kind="ExternalInput").ap()
    wvW = nc.dram_tensor("wvW", [128, 4096], FP16, kind="ExternalInput").ap()
    woW = nc.dram_tensor("woW", [128, 4096], FP16, kind="ExternalInput").ap()
    yW = nc.dram_tensor("yW", [128, 16384], FP16, kind="ExternalOutput").ap()

    with tile.TileContext(nc) as tc:
        with ExitStack() as ctx:
            if loop_k:
                ctx.enter_context(tc.For_i(0, loop_k, 1))
            _body(tc, xW, wqW, wkW, wvW, woW, yW, stages)
    nc.compile()
    return nc


def _body(tc, xW, wqW, wkW, wvW, woW, yW, stages="ABC"):
    nc = tc.nc
    use_B = "B" in stages
    use_C = "C" in stages
    with ExitStack() as ctx:
        ctx.enter_context(nc.allow_low_precision(reason="fp32r/fp16 pipeline"))
        pers = ctx.enter_context(tc.tile_pool(name="pers", bufs=1))
        qT = [pers.tile([128, T], FP16, tag=f"qT{m}", name=f"qT{m}")
              for m in range(NM)]
        kT = [pers.tile([128, T], FP16, tag=f"kT{m}", name=f"kT{m}")
              for m in range(NM)]
        # per-head V stride padded 65 -> 128 cols: the O matmuls' lhsT
        # slices start 256B-aligned (unaligned ldweights measured ~100ns/MM
        # slower on HW)
        Vs = [pers.tile([128, H * 128], FP16, tag=f"Vs{t}", name=f"Vs{t}")
              for t in range(NT)]
        # all 4 head-pairs' O^T partials in one allocation so the odd-head
        # partition-shift DMA can batch 4 heads per chunk into one transfer
        OTP = pers.tile([128, (H // 2) * T], FP16, tag="OTP", name="OTP")
        wpool = ctx.enter_context(tc.tile_pool(name="wpool", bufs=1))
        xpool = ctx.enter_context(tc.tile_pool(name="xpool", bufs=2))
        ppool = ctx.enter_context(tc.tile_pool(name="ppool", bufs=16))
        rpool = ctx.enter_context(tc.tile_pool(name="rpool", bufs=4))
        oupool = ctx.enter_context(tc.tile_pool(name="oupool", bufs=4))
        psS2 = ctx.enter_context(tc.tile_pool(name="psS2", bufs=2,
                                              space="PSUM"))
        psA = ctx.enter_context(tc.tile_pool(name="psA", bufs=2, space="PSUM"))
        psO = ctx.enter_context(tc.tile_pool(name="psO", bufs=2, space="PSUM"))
        ypool = ctx.enter_context(tc.tile_pool(name="ypool", bufs=2))

        # one big DMA per weight tensor (128 descriptors x 8KB, near-peak)
        wq = wpool.tile([128, 4096], FP16, tag="wq", name="wq")
        nc.sync.dma_start(wq[:], wqW)
        xts = {}
        xts[0] = xpool.tile([128, 4096], FP16, tag="xt", name="xt0")
        nc.sync.dma_start(xts[0][:], xW[:, 0:4096])
        wk = wpool.tile([128, 4096], FP16, tag="wk", name="wk")
        nc.sync.dma_start(wk[:], wkW)
        wv = wpool.tile([128, 4096], FP16, tag="wv", name="wv")
        nc.sync.dma_start(wv[:], wvW)
        Wo = wpool.tile([128, 4096], FP16, tag="Wo", name="Wo")
        nc.sync.dma_start(Wo[:], woW)

        def emit_evac_norm(O_ps, h, j):
            # evacuate the O psum IMMEDIATELY (psO recycles at copy speed,
            # not at normalize-chain speed), then normalize from SBUF:
            # 1/denominator (DVE), broadcast across the 64 feature
            # partitions (GpSimd), scale + write into OTP (DVE, using its
            # cross-partition base offset for odd heads)
            hp = h // 2
            Ou = oupool.tile([65, 512], FP16, tag="Ou", name="Ou")
            nc.vector.tensor_copy(Ou[:], O_ps[:])
            rt = rpool.tile([1, 512], FP16, tag="r", name="rt")
            nc.vector.reciprocal(rt[0:1, :], Ou[64:65, :])
            Rs = rpool.tile([64, 512], FP16, tag="Rs", name="Rs")
            nc.gpsimd.partition_broadcast(Rs[:], rt[0:1, :])
            cols = slice(T * hp + 512 * j, T * hp + 512 * j + 512)
            pb = 64 * (h % 2)
            nc.vector.tensor_mul(OTP[pb:pb + 64, cols], Ou[0:64, :], Rs[:])

        # ---- stage A emitters: per chunk j, 12 matmul groups ----
        def a_group(j, grp):
            if j not in xts:
                xt_ = xpool.tile([128, 4096], FP16, tag="xt", name=f"xt{j}")
                nc.sync.dma_start(xt_[:], xW[:, 4096 * j:4096 * j + 4096])
                xts[j] = xt_
            xt = xts[j]
            tsl = slice(512 * j, 512 * j + 512)
            if grp < 8:
                dst, w, m = ((qT, wq, grp) if grp < 4 else (kT, wk, grp - 4))
                ps = psA.tile([128, 512], F32, tag="A", name="psA")
                for k in range(NC):
                    nc.tensor.matmul(
                        ps[:], w[:, 512 * k + 128 * m:512 * k + 128 * m + 128],
                        xt[:, 512 * k:512 * k + 512], start=(k == 0),
                        stop=(k == NC - 1))
                nc.vector.tensor_copy(dst[m][:, tsl], ps[:])
            else:
                tt = grp - 8
                t_idx = 4 * j + tt
                ps = psA.tile([128, 512], F32, tag="A", name="psV")
                for k in range(NC):
                    nc.tensor.matmul(
                        ps[:], xt[:, 512 * k + 128 * tt:512 * k + 128 * tt + 128],
                        wv[:, 512 * k:512 * k + 512], start=(k == 0),
                        stop=(k == NC - 1))
                dst_ap = Vs[t_idx][:].rearrange("p (h e) -> p h e", e=128)
                nc.vector.tensor_copy(
                    dst_ap[:, :, 0:64],
                    ps[:].rearrange("p (h e) -> p h e", e=64))
                nc.vector.memset(dst_ap[:, :, 64:65], 1.0)

        y_done = {}

        def c_tile(m, n):
            j = m // 4
            y_ps = psA.tile([128, 512], F32, tag="A", name="yps")
            for p in range(H // 2):
                nc.tensor.matmul(
                    y_ps[:], OTP[:, T * p + 128 * m:T * p + 128 * m + 128],
                    Wo[:, 1024 * p + 512 * n:1024 * p + 512 * n + 512],
                    start=(p == 0), stop=(p == H // 2 - 1))
            if j not in y_done:
                y_done[j] = [ypool.tile([128, 4096], FP16, tag="y",
                                        name=f"ysb{j}"), 0]
            ysb, _ = y_done[j]
            mo = m - 4 * j
            nc.vector.tensor_copy(
                ysb[:, 1024 * mo + 512 * n:1024 * mo + 512 * n + 512], y_ps[:])
            y_done[j][1] += 1
            if y_done[j][1] == 8:
                nc.gpsimd.dma_start(yW[:, 4096 * j:4096 * j + 4096], ysb[:])

        a_todo = {j: list(range(12)) for j in range(NJ)}
        c_todo = []
        c_wait = {j: [(m, n) for m in range(4 * j, 4 * j + 4)
                      for n in range(NYN)] for j in range(NJ)}
        norm_done = {j: 0 for j in range(NJ)}

        def emit_filler(j_next, k):
            # prefer A-groups for the next chunk, then ready C-tiles
            while k > 0:
                if a_todo.get(j_next):
                    a_group(j_next, a_todo[j_next].pop(0))
                elif c_todo:
                    c_tile(*c_todo.pop(0))
                else:
                    return
                k -= 1

        for g in range(12):
            a_group(0, a_todo[0].pop(0))

        def finish_head(O_ps, h, j):
            emit_evac_norm(O_ps, h, j)
            norm_done[j] += 1
            if use_C and norm_done[j] == H:
                c_todo.extend(c_wait.pop(j))

        for j in (range(NJ) if use_B else []):
            for hp in range(H // 2):
                h2 = (2 * hp, 2 * hp + 1)
                kTh, qTh = kT[hp], qT[hp]
                i_max = DJ * j + DJ - 1
                Ps = []
                for i in range(i_max + 1):
                    mloc = i - DJ * j
                    off = 128 * mloc if mloc > 0 else 0
                    w = 512 - off
                    S2 = psS2.tile([128, 1024], F32, tag="S2", name="S2")
                    diag = mloc >= 0
                    for idx in range(2):
                        pb = 64 * idx
                        nc.tensor.matmul(
                            S2[:, 512 * idx + off:512 * idx + off + w],
                            kTh[pb:pb + 64, 128 * i:128 * i + 128],
                            qTh[pb:pb + 64, 512 * j + off:512 * j + off + w],
                            start=True, stop=True)
                    P2 = ppool.tile([128, 1024], FP16, tag="P", name="P")
                    nc.scalar.activation(P2[:, 0:1024 - off],
                                         S2[:, off:1024], EXP, scale=0.125)
                    if diag:
                        # causal mask: zero P2 where tk > tq in the diagonal
                        # 128-block of both heads (cols 0:128 and 512:640),
                        # on the otherwise-idle GpSimd engine
                        P2r = P2[:].rearrange("p (b c) -> p b c",
                                              b=2)[:, :, 0:128]
                        nc.gpsimd.affine_select(
                            out=P2r, in_=P2r,
                            compare_op=mybir.AluOpType.is_ge,
                            fill=0.0, base=0,
                            pattern=[[0, 2], [1, 128]],
                            channel_multiplier=-1)
                    Ps.append((P2, off, w))
                    if i % 2 == 1:
                        emit_filler(j + 1, 1)
                emit_filler(j + 1, 2)
                for idx in (0, 1):
                    O_ps = psO.tile([65, 512], F32, tag="O", name="Ops")
                    h = h2[idx]
                    for i, (P2, off, w) in enumerate(Ps):
                        rhs = (P2[:, 0:w] if idx == 0
                               else P2[:, 512:512 + w])
                        nc.tensor.matmul(
                            O_ps[:, off:off + w],
                            Vs[i][:, 128 * h:128 * h + 65], rhs,
                            start=(i == 0), stop=(i == i_max))
                    finish_head(O_ps, h, j)
        if use_C:
            while c_todo or any(c_wait.values()):
                if not c_todo:
                    for j in sorted(list(c_wait)):
                        c_todo.extend(c_wait.pop(j))
                c_tile(*c_todo.pop(0))
        else:
            dummy = ypool.tile([128, 4096], FP16, tag="y", name="ydum")
            nc.vector.memset(dummy[:, 0:512], 0.0)
            nc.sync.dma_start(yW[:, 0:512], dummy[:, 0:512])


_NC_CACHE = {}


def _get_nc(loop_k=0, stages="ABC"):
    key = (loop_k, stages)
    if key not in _NC_CACHE:
        _NC_CACHE[key] = build_nc(loop_k, stages)
    return _NC_CACHE[key]


def make_in_maps(x, Wq, Wk, Wv, Wo):
    x = np.asarray(x, dtype=np.float32)
    Wq = np.asarray(Wq, dtype=np.float32)
    Wk = np.asarray(Wk, dtype=np.float32)
    Wv = np.asarray(Wv, dtype=np.float32)
    Wo = np.asarray(Wo, dtype=np.float32)

    def relay_w(Wslice):
        # [512, 1024] -> wT [1024, 512] -> [8(k), 128(p), 512] -> [p, k*512]
        wT = Wslice.T.reshape(8, 128, 512).transpose(1, 0, 2).reshape(128, 4096)
        return np.ascontiguousarray(wT).astype(np.float16)

    def relay_wo(Wslice):
        # Wo[:, sl].T = [512, 1024] -> [4(g), 128(p), 1024] -> [p, g*1024]
        wT = Wslice.T.reshape(4, 128, 1024).transpose(1, 0, 2).reshape(128, 4096)
        return np.ascontiguousarray(wT).astype(np.float16)

    xWs = []
    for b in range(B):
        # xT [1024(c), 2048(t)] -> [8(k),128(p),4(j),512(tc)] -> [p, j, k, tc]
        xT = x[b].T.reshape(8, 128, 4, 512).transpose(1, 2, 0, 3)
        xWs.append(np.ascontiguousarray(xT.reshape(128, 16384))
                   .astype(np.float16))

    in_maps = []
    for c in range(8):
        b, g = c // 2, c % 2
        sl = slice(HD * g, HD * g + HD)
        in_maps.append({
            "xW": xWs[b],
            "wqW": relay_w(Wq[sl, :]),
            "wkW": relay_w(Wk[sl, :]),
            "wvW": relay_w(Wv[sl, :]),
            "woW": relay_wo(Wo[:, sl]),
        })
    return in_maps


def kernel(x, Wq, Wk, Wv, Wo):
    nc = _get_nc()
    in_maps = make_in_maps(x, Wq, Wk, Wv, Wo)
    res = run_bass_kernel_spmd(nc, in_maps, core_ids=list(range(8)))
    out = np.empty((B, T, C), dtype=np.float32)
    for b in range(B):
        # yW [128, 16*1024] -> y [2048, 1024]
        ys = []
        for cid in (2 * b, 2 * b + 1):
            yw = res.results[cid]["yW"].astype(np.float32)
            ys.append(yw.reshape(128, 16, 1024).transpose(1, 0, 2)
                      .reshape(T, C))
        out[b] = ys[0] + ys[1]
    return out
